# revision 35
# baseline (speedup 1.0000x reference)
"""MultiHeadGAT layer on 8 trn2 NeuronCores, data-parallel over batch.

Per core (one batch element), with softmax-invariant rescaling: dividing the
unnormalized attention P[j,i] = exp(leaky_relu(e_src[i]+e_dst[j])) by
exp(e_src[i]) (a per-i factor that cancels in the softmax) gives

  P'[j,i] = max( exp(-0.8*e_src[i]) * exp(0.2*e_dst[j]),  exp(e_dst[j]) )

i.e. ONE fused DVE tensor_scalar op per [128,1024] tile (mult + max against
two per-partition scalars) in bf16 -- no exps in the main loop at all.
Mask multiply runs as one [128,2048] bf16 tensor_tensor per HEAD PAIR
(adjacency duplicated along the free dim), all on DVE (gpsimd contends for
SBUF ports and slows DVE ~4x -- measured).  AV matmul in bf16 (1 cycle/row)
with a ones column appended to the lhsT so row 64 of the accumulator is the
softmax denominator.

Schedule notes (all measured on HW):
 - e_src broadcast: heads 0-3 via PE one-hot-selector matmuls (low latency),
   heads 4-7 via four parallel in-SBUF DMA log-doubling chains (a chain hop
   costs ~2.5us of completion-semaphore latency, so chains only suit heads
   needed >35us in).
 - Per-pair epilogue's DVE ops (reciprocal of the transposed denominators,
   last pair's divide) are DEFERRED past the next pair's first tiles so the
   in-order DVE queue never stalls on the acc->ACT-copy->PE-transpose chain;
   earlier pairs divide on ACT.
 - Output staged in one [128, cb, 512] tile laid out so each pair's quarter
   flushes as a single DMA; the host undoes the permutation with a reshape.
 - DMA issues cost ~650ns each on the Sync queue: inputs are packed into one
   [256, 1552] tensor ([h.T | W@A | W], 2 DMAs) plus 8 adjacency-pair DMAs.

Host-side prep (layout/dtype only): h.T, adj.T (duplicated) as bf16, W and
W@A as bf16, output un-permutation.  ~96us HW, vs 227us for the fp32
ACT-exp/DVE-max baseline: PE was 99% busy there (fp32 matmul = 4 cyc/row,
issued as 2 HW instrs), and ACT's 2 full exps/element had no bf16 speedup.
"""
import sys

sys.path.insert(0, "/opt/trn_rl_repo")

import numpy as np
import ml_dtypes

import concourse.bass as bass
import concourse.mybir as mybir
import concourse.tile as tile
from concourse.bass_utils import run_bass_kernel_spmd
from concourse.masks import make_identity

F32 = mybir.dt.float32
BF16 = mybir.dt.bfloat16
AF = mybir.ActivationFunctionType
ALU = mybir.AluOpType
BF16NP = ml_dtypes.bfloat16

N_CORES = 8
N = 1024
NB = 8          # row blocks of 128
FIN = 256
KT = 2          # FIN / 128
FO = 512        # heads * fo
H = 8
FOH = 64
ALPHA = 0.2

NSEL = 4        # heads 0-3 via PE selector; 4-7 via parallel DMA doubling

_MAX_SYNC_WAITS = 1


def _split_sync_waits(nc, max_waits=_MAX_SYNC_WAITS):
    """This walrus build rejects instructions carrying more than one sync
    wait; hoist extras onto NOPs inserted just before, on the same engine."""
    uid = 0
    for f in nc.m.functions:
        for bb in f.blocks:
            out = []
            for inst in bb.instructions:
                si = getattr(inst, "sync_info", None)
                if si is not None and si.on_wait and len(si.on_wait) > max_waits:
                    waits = list(si.on_wait)
                    keep = waits[-max_waits:]
                    extra = waits[:-max_waits]
                    si.on_wait.clear()
                    si.on_wait.extend(keep)
                    while extra:
                        chunk, extra = extra[:max_waits], extra[max_waits:]
                        nop = mybir.InstNoOp(
                            name=f"waitsplit-{uid}",
                            engine=inst.engine,
                            sync_info=mybir.SyncInfo(
                                on_wait=list(chunk), on_update=[]
                            ),
                            bass_nofuse=True,
                        )
                        uid += 1
                        out.append(nop)
                out.append(inst)
            bb.instructions[:] = out


def build_nc(split=True):
    nc = bass.Bass()
    PREW = N + 2 * H + FO   # hTb | WAb | Wb packed along the free dim
    pre_d = nc.declare_dram_parameter("pre", [FIN, PREW], BF16, isOutput=False)
    adjT_d = nc.declare_dram_parameter("adjT2", [N, 2 * N], BF16, isOutput=False)
    # out stored [p, cb, hcol]: row cb*128+p of the logical output lives at
    # out_d[p, cb, :]; the host undoes this with a reshape/transpose
    out_d = nc.declare_dram_parameter("out", [128, NB, FO], F32, isOutput=True)

    with tile.TileContext(nc) as tc:
        with (
            tc.tile_pool(name="const", bufs=1) as const,
            tc.tile_pool(name="persist", bufs=1) as persist,
            tc.tile_pool(name="tp8", bufs=10) as tpool,
            tc.tile_pool(name="epi", bufs=4) as epi,
            tc.tile_pool(name="psS", bufs=4, space="PSUM") as psS,
            tc.tile_pool(name="psAcc", bufs=1, space="PSUM") as psAcc,
        ):
            ident = const.tile([128, 128], F32, tag="ident")
            make_identity(nc, ident[:])

            pre = [persist.tile([128, PREW], BF16, tag=f"pre{k}",
                                name=f"pre{k}")
                   for k in range(KT)]
            for k in range(KT):
                nc.sync.dma_start(pre[k][:], pre_d[k * 128:(k + 1) * 128, :])
            hT = [pre[k][:, 0:N] for k in range(KT)]
            wa = [pre[k][:, N:N + 2 * H] for k in range(KT)]
            wk = [pre[k][:, N + 2 * H:PREW] for k in range(KT)]
            # adjacency duplicated along free dim (host-prepped): one TT
            # masks a head pair
            adjT2 = [persist.tile([128, 2 * N], BF16, tag=f"adjT{j}",
                                  name=f"adjT{j}")
                     for j in range(NB)]
            for jb in range(NB):
                nc.sync.dma_start(
                    adjT2[jb][:], adjT_d[jb * 128:(jb + 1) * 128, :]
                )

            # ---- E_T[16, i] = (WA.T @ hT): rows 0..7 e_src, 8..15 e_dst;
            # G8 = exp(-(1-alpha)*e_src) read straight from PSUM.  Two
            # half-tiles so jb<4 transposes only wait on the c=0 half. ----
            e_tc = [const.tile([16, 512], F32, tag=f"eT{c}", name=f"eT{c}")
                    for c in range(2)]
            g8 = const.tile([8, N], BF16, tag="g8")
            for c in range(2):
                ps = psS.tile([16, 512], F32, tag="ps")
                for k in range(KT):
                    nc.tensor.matmul(
                        ps[:], wa[k], hT[k][:, c * 512:(c + 1) * 512],
                        start=(k == 0), stop=(k == KT - 1),
                    )
                nc.scalar.activation(
                    g8[:, c * 512:(c + 1) * 512], ps[0:8, :], AF.Exp,
                    scale=-(1.0 - ALPHA),
                )
                nc.vector.tensor_copy(e_tc[c][:], ps[:])

            # ---- e_sb[jb][p, 16] = E_T[:, jb*128+p]; s0/s1 = per-j scalars ----
            e_sb = [persist.tile([128, 16], F32, tag=f"E{j}", name=f"E{j}")
                    for j in range(NB)]
            s0sb = [persist.tile([128, H], F32, tag=f"s0{j}", name=f"s0{j}")
                    for j in range(NB)]
            s1sb = [persist.tile([128, H], F32, tag=f"s1{j}", name=f"s1{j}")
                    for j in range(NB)]
            def esb(jb):
                tp = psS.tile([128, 512], F32, tag="ps")
                nc.tensor.transpose(
                    tp[:, 0:16],
                    e_tc[jb // 4][:, (jb % 4) * 128:(jb % 4 + 1) * 128],
                    ident[0:16, 0:16],
                )
                nc.vector.tensor_copy(e_sb[jb][:], tp[:, 0:16])

            esb(0)

            def late_esb():
                for jb in range(2, NB):
                    esb(jb)

            # ---- Gb broadcast over partitions via PE selector matmuls.
            # Emission order feeds pair 0 first: heads 0-1, then jb=0 s-cols,
            # then the rest -- PE and ACT are otherwise idle here. ----
            gbsel = [persist.tile([128, N], BF16, tag=f"gb{hh}", name=f"gb{hh}")
                     for hh in range(H)]
            sel = []
            for hh in range(NSEL):
                t = const.tile([8, 128], BF16, tag=f"sel{hh}", name=f"sel{hh}")
                nc.gpsimd.memset(t[:], 0.0)
                nc.gpsimd.affine_select(
                    out=t[:], in_=t[:], pattern=[[0, 128]],
                    compare_op=ALU.not_equal, fill=1.0,
                    base=-hh, channel_multiplier=1,
                )
                sel.append(t)

            def bcast_head(hh, split=False):
                # split=True: c=1 copy on DVE (idle during the prologue) so
                # the two psum->SBUF copies run in parallel with ACT's
                for c in range(2):
                    ps = psS.tile([128, 512], F32, tag="ps")
                    nc.tensor.matmul(
                        ps[:], sel[hh][:], g8[:, c * 512:(c + 1) * 512],
                        start=True, stop=True,
                    )
                    if split and c == 1:
                        nc.vector.tensor_copy(
                            gbsel[hh][:, c * 512:(c + 1) * 512], ps[:]
                        )
                    else:
                        nc.scalar.copy(
                            gbsel[hh][:, c * 512:(c + 1) * 512], ps[:]
                        )

            def scols(jb):
                # s0 = exp(alpha * e_dst), s1 = exp(e_dst)
                nc.scalar.activation(
                    s0sb[jb][:], e_sb[jb][:, 8:16], AF.Exp, scale=ALPHA,
                )
                nc.scalar.activation(
                    s1sb[jb][:], e_sb[jb][:, 8:16], AF.Exp, scale=1.0,
                )

            scols(0)
            bcast_head(0)
            esb(1)
            scols(1)
            bcast_head(1)
            late_esb()
            for hh in range(NSEL, H):
                t = gbsel[hh]
                nc.sync.dma_start(t[0:1, :], g8[hh:hh + 1, :])
                p = 1
                while p < 128:
                    nc.sync.dma_start(t[p:2 * p, :], t[0:p, :])
                    p *= 2
            for jb in range(2, NB):
                scols(jb)

            def gb(hh):
                return gbsel[hh][:, :]

            # ---- wh_aug[jb][j, h, 0:64] = (h @ W) block bf16, [:, h, 64] = 1 ----
            wh_aug = [persist.tile([128, H, 65], BF16, tag=f"wha{j}",
                                   name=f"wha{j}")
                      for j in range(NB)]
            for jb in range(NB):
                ps = psS.tile([128, H, FOH], F32, tag="ps")
                for k in range(KT):
                    nc.tensor.matmul(
                        ps[:, :, :], hT[k][:, jb * 128:(jb + 1) * 128], wk[k],
                        start=(k == 0), stop=(k == KT - 1),
                    )
                nc.scalar.activation(
                    wh_aug[jb][:, :, 0:64], ps[:, :, :], AF.Copy,
                )
                nc.gpsimd.memset(wh_aug[jb][:, :, 64:65], 1.0)
            for hh in range(2, NSEL):
                bcast_head(hh)

            # ---- output staging: osm_big[p, cb, h*64+f] ----
            osm_big = persist.tile([128, NB, FO], F32, tag="osm")

            # ---- main attention loop, head pairs ----
            # Epilogue DVE work (recip + last-pair osm) is deferred until the
            # next pair's first jb tiles are queued, so the in-order DVE queue
            # never stalls on the acc->ACT->PE transpose chain.
            pending = [None]

            def emit_pending():
                if pending[0] is not None:
                    pending[0]()
                    pending[0] = None

            for hp in range(H // 2):
                h0, h1 = 2 * hp, 2 * hp + 1
                acc = {
                    (hh, c): psAcc.tile([65, 512], F32, tag=f"acc{hh % 2}{c}",
                                        name=f"acc{hh % 2}{c}")
                    for hh in (h0, h1) for c in range(2)
                }
                t2s = {}

                def tsp(jb, q, hh):
                    nc.vector.tensor_scalar(
                        t2s[jb][:, q * N:(q + 1) * N], gb(hh),
                        s0sb[jb][:, hh:hh + 1], s1sb[jb][:, hh:hh + 1],
                        ALU.mult, ALU.max,
                    )

                if hp == 0:
                    # h1's broadcast lands ~1us after h0's: fill DVE with
                    # h0's first two tiles meanwhile
                    for jb in range(2):
                        t2s[jb] = tpool.tile([128, 2 * N], BF16, tag="t2",
                                             name=f"t2w{jb}")
                        tsp(jb, 0, h0)
                for jb in range(NB):
                    if jb not in t2s:
                        t2s[jb] = tpool.tile([128, 2 * N], BF16, tag="t2",
                                             name=f"t2_{jb}")
                        tsp(jb, 0, h0)
                    t2 = t2s[jb]
                    tsp(jb, 1, h1)
                    nc.vector.tensor_mul(t2[:], t2[:], adjT2[jb][:])
                    for q, hh in enumerate((h0, h1)):
                        for c in range(2):
                            nc.tensor.matmul(
                                acc[(hh, c)][:],
                                wh_aug[jb][:, hh, :],
                                t2[:, q * N + c * 512:q * N + (c + 1) * 512],
                                start=(jb == 0), stop=(jb == NB - 1),
                            )
                    if jb == 2:
                        emit_pending()
                # epilogue: acc -> SBUF (ACT) + transposes now; DVE deferred
                tps = {}
                recs = {}
                for hh in (h0, h1):
                    acc_sb = epi.tile([65, N], F32, tag="accsb")
                    recs[hh] = epi.tile([128, 8], F32, tag="rec8",
                                        name=f"rec8_{hh}")
                    for q in range(2):
                        nc.scalar.copy(
                            acc_sb[:, q * 512:(q + 1) * 512], acc[(hh, q)][:]
                        )
                        tp = psS.tile([128, 4 * 65], F32, tag="ps")
                        for r in range(4):
                            cb = q * 4 + r
                            nc.tensor.transpose(
                                tp[:, r * 65:r * 65 + 65],
                                acc_sb[:, cb * 128:(cb + 1) * 128],
                                ident[0:65, 0:65],
                            )
                        tps[(hh, q)] = tp

                def emit_epilogue(hp=hp, h0=h0, h1=h1, tps=tps, recs=recs):
                    last = hp == H // 2 - 1
                    for hh in (h0, h1):
                        rec8 = recs[hh]
                        for q in range(2):
                            tp = tps[(hh, q)]
                            nc.vector.reciprocal(
                                rec8[:, q * 4:(q + 1) * 4], tp[:, 64::65]
                            )
                            for r in range(4):
                                cb = q * 4 + r
                                if last and q == 0:
                                    # final pair: q0 half on DVE, q1 on ACT
                                    nc.vector.tensor_scalar(
                                        osm_big[:, cb,
                                                hh * FOH:(hh + 1) * FOH],
                                        tp[:, r * 65:r * 65 + 64],
                                        rec8[:, cb:cb + 1], None, ALU.mult,
                                    )
                                else:
                                    nc.scalar.activation(
                                        osm_big[:, cb,
                                                hh * FOH:(hh + 1) * FOH],
                                        tp[:, r * 65:r * 65 + 64], AF.Copy,
                                        scale=rec8[:, cb:cb + 1],
                                    )
                            if last:
                                # stream out each 4-block half as it lands
                                nc.sync.dma_start(
                                    out_d[:, q * 4:(q + 1) * 4,
                                          hh * FOH:(hh + 1) * FOH],
                                    osm_big[:, q * 4:(q + 1) * 4,
                                            hh * FOH:(hh + 1) * FOH],
                                )
                    # single-DMA flush of this pair's 128-col quarter
                    # (the last pair streamed per half-block above)
                    if not last:
                        nc.sync.dma_start(
                            out_d[:, :, hp * 128:(hp + 1) * 128],
                            osm_big[:, :, hp * 128:(hp + 1) * 128],
                        )

                if hp == H // 2 - 1:
                    emit_epilogue()
                else:
                    pending[0] = emit_epilogue

    if split:
        _split_sync_waits(nc)
    return nc


_NC_CACHE = None


def _get_nc():
    global _NC_CACHE
    if _NC_CACHE is None:
        _NC_CACHE = build_nc()
    return _NC_CACHE


def _dup_adjT(adj_c):
    at = np.ascontiguousarray(adj_c.T).astype(BF16NP)
    return np.ascontiguousarray(np.concatenate([at, at], axis=1))


def _prep_in_maps(h, adj, W, a):
    h = np.ascontiguousarray(h, dtype=np.float32)
    adj = np.ascontiguousarray(adj, dtype=np.int32)
    W = np.ascontiguousarray(W, dtype=np.float32)
    a = np.ascontiguousarray(a, dtype=np.float32)
    amat = np.zeros((FO, 2 * H), dtype=np.float32)
    for hh in range(H):
        amat[hh * FOH:(hh + 1) * FOH, hh] = a[hh, :FOH]
        amat[hh * FOH:(hh + 1) * FOH, H + hh] = a[hh, FOH:]
    wamat = (W @ amat).astype(BF16NP)
    wb = W.astype(BF16NP)
    return [
        {
            "pre": np.ascontiguousarray(np.concatenate(
                [h[c].T.astype(BF16NP), wamat, wb], axis=1)),
            "adjT2": _dup_adjT(adj[c]),
        }
        for c in range(N_CORES)
    ]


def run(h, adj, W, a, trace=False, **kw):
    nc = _get_nc()
    in_maps = _prep_in_maps(h, adj, W, a)
    res = run_bass_kernel_spmd(nc, in_maps, list(range(N_CORES)), trace=trace, **kw)
    out = np.stack(
        [res.results[c]["out"].transpose(1, 0, 2).reshape(N, FO)
         for c in range(N_CORES)], axis=0)
    return out.astype(np.float32), res


def kernel(h, adj, W, a):
    out, _ = run(h, adj, W, a)
    return out
